# revision 1
# baseline (speedup 1.0000x reference)
"""Causal self-attention (B=2, T=4096, C=768, H=12) on 8 trn2 NeuronCores.

Sharding: data-parallel on batch (cores 0-3 -> batch 0, cores 4-7 -> batch 1),
tensor-parallel on heads (3 heads per core).  Each core computes qkv for its
3 heads, causal flash-style attention, and a partial output projection
(its heads' rows of w_proj); the host sums the 4 partials per batch.

v11 structure (vs the serial-phase v7 baseline, ~2.05x faster: 676us ->
330us per iteration measured via repeat-differencing with block sampling):
- All activations/weights in bf16 (host-converted): halves DMA traffic and
  removes every fp32->fp32r rounding copy.  PSUM accumulation stays fp32.
  Partial Y outputs are written bf16 and summed fp32 on the host.
- Causal masking via gpsimd affine_select directly on the exp'd P tile
  (Pool engine is otherwise idle), freeing DVE; diagonal tiles compute
  S/PV ragged (columns left of the diagonal tile are skipped).
- One software-pipelined loop: the qkv projection chunk qs+1, V^T->V
  transposes, and the output projection for qs-1 are emitted interleaved
  into the attention rotation for query superblock qs, so their DMA/PE/
  DVE work hides under the attention inner loop (PE ~83% busy in sim).
- x^T is host-swizzled to [partition, chunk, cchunk, token] so each chunk
  DMA is one contiguous 6KB run per partition; y writes batch 4 token
  tiles per DMA.
"""

import sys

if '/opt/trn_rl_repo' not in sys.path:
    sys.path.insert(0, '/opt/trn_rl_repo')

from collections import deque

import numpy as np
import ml_dtypes

import concourse.bacc as bacc
import concourse.mybir as mybir
import concourse.tile as tile
from concourse.masks import make_identity

dt = mybir.dt
F32 = dt.float32
BF16 = dt.bfloat16
NP_BF16 = ml_dtypes.bfloat16

N_EMBD = 768
N_HEADS = 12
HEAD_DIM = 64
B = 2
T_FULL = 4096
N_CORES = 8
HEADS_PER_CORE = N_HEADS // (N_CORES // B)  # 3

TOK_CHUNK = 512   # qkv phase token chunk == query superblock
QSB = 512         # attention query superblock
KT = 128          # key tile (contraction for P@V)
CCHUNKS = N_EMBD // 128  # 6 contraction chunks


def build_nc(T=T_FULL, repeat=1, phases=('B', 'B2', 'C', 'D')):
    """Build the per-core Bass program.  Same program runs SPMD on all 8
    cores; per-core data (x^T of its batch, its heads' weight slices) comes
    via the input map.  `phases` subsets the per-iteration work (timing
    ablation only -- outputs are garbage unless all phases run)."""
    nc = bacc.Bacc(None, target_bir_lowering=False, debug=False)

    n_kt = T // KT
    n_qsb = T // QSB
    n_tok = T // 128
    kt_per_qsb = QSB // KT  # 4

    # x^T pre-swizzled on host to [p, chunk, cchunk, tok]: each chunk DMA
    # reads one contiguous 6KB run per partition.
    XT = nc.dram_tensor(
        "xt", [128, T // TOK_CHUNK, CCHUNKS, TOK_CHUNK], BF16,
        kind="ExternalInput")
    WQ01 = nc.dram_tensor("wq01", [N_EMBD, 128], BF16, kind="ExternalInput")
    WK01 = nc.dram_tensor("wk01", [N_EMBD, 128], BF16, kind="ExternalInput")
    WV01 = nc.dram_tensor("wv01", [N_EMBD, 128], BF16, kind="ExternalInput")
    WQV2 = nc.dram_tensor("wqv2", [N_EMBD, 128], BF16, kind="ExternalInput")
    WK2 = nc.dram_tensor("wk2", [N_EMBD, 64], BF16, kind="ExternalInput")
    WP1 = nc.dram_tensor("wp1", [128, N_EMBD], BF16, kind="ExternalInput")
    WP2 = nc.dram_tensor("wp2", [64, N_EMBD], BF16, kind="ExternalInput")
    Y = nc.dram_tensor("y", [T, N_EMBD], BF16, kind="ExternalOutput")

    xt_ap = XT.ap()

    with tile.TileContext(nc) as tc:
        with (
            tc.tile_pool(name="const", bufs=1) as const_pool,
            tc.tile_pool(name="wpool", bufs=1) as wpool,
            tc.tile_pool(name="qkvt", bufs=1) as qkvt,
            tc.tile_pool(name="vsb", bufs=1) as vsb_pool,
            tc.tile_pool(name="ynt", bufs=1) as ynt_pool,
            tc.tile_pool(name="xs", bufs=3) as xs_pool,
            tc.tile_pool(name="ptp", bufs=4) as pt_pool,
            tc.tile_pool(name="ysb", bufs=3) as ysb_pool,
            tc.tile_pool(name="rp", bufs=8) as r_pool,
            tc.tile_pool(name="yout", bufs=3) as yout_pool,
            tc.tile_pool(name="yqn", bufs=4) as yqn_pool,
            tc.tile_pool(name="pbig", bufs=2, space="PSUM") as pbig,
            tc.tile_pool(name="py", bufs=2, space="PSUM") as py_pool,
            tc.tile_pool(name="paux", bufs=2, space="PSUM") as paux,
        ):
            # ---- weights: direct bf16 DMA (first, so phase B isn't gated
            # on constant construction; spread across two idle queues) ----
            _weng = [nc.gpsimd, nc.scalar]

            def load_w(src_ap, shape, tag, i=[0]):
                t = wpool.tile(shape, BF16, tag=tag)
                _weng[i[0] % 2].dma_start(out=t, in_=src_ap)
                i[0] += 1
                return t

            wq01r = load_w(WQ01.ap().rearrange("(c p) m -> p c m", p=128), [128, CCHUNKS, 128], "wq01r")
            wk01r = load_w(WK01.ap().rearrange("(c p) m -> p c m", p=128), [128, CCHUNKS, 128], "wk01r")
            wv01r = load_w(WV01.ap().rearrange("(c p) m -> p c m", p=128), [128, CCHUNKS, 128], "wv01r")
            wqv2r = load_w(WQV2.ap().rearrange("(c p) m -> p c m", p=128), [128, CCHUNKS, 128], "wqv2r")
            wk2r = load_w(WK2.ap().rearrange("(c p) m -> p c m", p=128), [128, CCHUNKS, 64], "wk2r")
            wp1r = load_w(WP1.ap(), [128, N_EMBD], "wp1r")
            wp2r = load_w(WP2.ap(), [64, N_EMBD], "wp2r")

            # ---- constants ----
            ident_f = const_pool.tile([128, 128], F32)
            make_identity(nc, ident_f)
            identb = const_pool.tile([128, 128], BF16)
            nc.vector.tensor_copy(out=identb, in_=ident_f)

            # ---- persistent activations ----
            QT01 = qkvt.tile([128, T], BF16, tag="qt01")
            KT01 = qkvt.tile([128, T], BF16, tag="kt01")
            VT01 = qkvt.tile([128, T], BF16, tag="vt01")
            QV2 = qkvt.tile([128, T], BF16, tag="qv2")   # q_h2 rows 0:64, v_h2 rows 64:128
            KT2 = qkvt.tile([64, T], BF16, tag="kt2")
            Vsb = vsb_pool.tile([128, n_kt, HEADS_PER_CORE, 65], BF16)
            YnT01 = ynt_pool.tile([128, T], BF16, tag="ynt01")
            YnT2 = ynt_pool.tile([64, T], BF16, tag="ynt2")

            ones_f = const_pool.tile([128, n_kt * HEADS_PER_CORE], F32)
            nc.vector.memset(ones_f, 1.0)
            nc.vector.tensor_copy(
                out=Vsb[:, :, :, 64:65].rearrange("p a b c -> p (a b c)"),
                in_=ones_f)

            if phases != ('B', 'B2', 'C', 'D'):
                # timing-ablation build: zero every cross-phase tensor once so
                # skipped producers leave consumers with defined data
                for t in (QT01, KT01, VT01, QV2, YnT01):
                    nc.vector.memset(t, 0.0)
                for t in (KT2, YnT2):
                    nc.vector.memset(t, 0.0)
                nc.vector.memset(Vsb[:, :, :, 0:64], 0.125)

            qkv_jobs = [
                (wq01r, QT01, 128), (wk01r, KT01, 128), (wv01r, VT01, 128),
                (wqv2r, QV2, 128), (wk2r, KT2, 64),
            ]

            for _ in range(repeat):
                # ---------- work generators ----------
                def b_chunk_gen(ch, split_dma=False):
                    """qkv projection for token chunk ch ([512] tokens)."""
                    sl = slice(ch * TOK_CHUNK, (ch + 1) * TOK_CHUNK)
                    xs = xs_pool.tile([128, CCHUNKS, TOK_CHUNK], BF16)
                    if split_dma:
                        # halve time-to-first-matmul at program start
                        h = CCHUNKS // 2
                        nc.sync.dma_start(out=xs[:, 0:h], in_=xt_ap[:, ch, 0:h])
                        nc.sync.dma_start(out=xs[:, h:], in_=xt_ap[:, ch, h:])
                    else:
                        nc.sync.dma_start(out=xs, in_=xt_ap[:, ch])
                    yield
                    for wt, out_sb, m in qkv_jobs:
                        ps = paux.tile([128, TOK_CHUNK], F32, tag="aux")
                        for c in range(CCHUNKS):
                            nc.tensor.matmul(
                                ps[0:m, :], wt[:, c, 0:m], xs[:, c, :],
                                start=(c == 0), stop=(c == CCHUNKS - 1),
                            )
                        nc.vector.tensor_copy(out=out_sb[0:m, sl], in_=ps[0:m, :])
                        yield

                def b2_gen(ch):
                    """V^T -> V (keys-major) transposes for chunk ch's key
                    tiles.  Heads 0+1 ride one [128,128] transpose."""
                    for kt in range(ch * kt_per_qsb, (ch + 1) * kt_per_qsb):
                        ks = slice(kt * KT, (kt + 1) * KT)
                        pv = paux.tile([128, 128], BF16, tag="aux")
                        nc.tensor.transpose(pv, VT01[:, ks], identb)
                        nc.vector.tensor_copy(
                            out=Vsb[:, kt, 0:2, 0:64],
                            in_=pv.rearrange("p (b c) -> p b c", b=2))
                        yield
                        pv2 = paux.tile([128, 64], BF16, tag="aux")
                        nc.tensor.transpose(pv2, QV2[64:128, ks], identb[64:128, 64:128])
                        nc.vector.tensor_copy(out=Vsb[:, kt, 2, 0:64], in_=pv2)
                        yield

                def d_gen(qs):
                    """partial output projection for query superblock qs.
                    All four 128-token tiles stage into one buffer so the
                    write-back is a single [128, 4, 768] DMA."""
                    n_tt = QSB // 128
                    yo = yout_pool.tile([128, n_tt, N_EMBD], BF16)
                    for tt4 in range(n_tt):
                        tt = qs * n_tt + tt4
                        tsl = slice(tt * 128, (tt + 1) * 128)
                        for c0, ncols in ((0, 512), (512, 256)):
                            pp = paux.tile([128, 512], F32, tag="aux")
                            nc.tensor.matmul(pp[:, 0:ncols], YnT01[:, tsl],
                                             wp1r[:, c0:c0 + ncols], start=True, stop=False)
                            nc.tensor.matmul(pp[:, 0:ncols], YnT2[0:64, tsl],
                                             wp2r[0:64, c0:c0 + ncols], start=False, stop=True)
                            nc.vector.tensor_copy(out=yo[:, tt4, c0:c0 + ncols],
                                                  in_=pp[:, 0:ncols])
                            yield
                    nc.sync.dma_start(
                        out=Y.ap()[qs * QSB:(qs + 1) * QSB, :]
                            .rearrange("(tt p) c -> p tt c", p=128),
                        in_=yo)
                    yield

                # ---------- attention ----------
                head_qk = [
                    (QT01[0:64, :], KT01[0:64, :]),
                    (QT01[64:128, :], KT01[64:128, :]),
                    (QV2[0:64, :], KT2[0:64, :]),
                ]

                def attend_kloop_gen(h, qs, nkt_q, yps):
                    qt_h, kt_h = head_qk[h]
                    for kt2 in range(0, nkt_q, 2):
                        yield
                        # diagonal tiles (delta > 0): query columns < delta
                        # see none of the tile's keys, so S/PV run ragged
                        # starting at column max(q0, delta).  The exp still
                        # covers [q0:QSB] for the pair; garbage columns in
                        # pt2 are never read by the narrowed PV.
                        last = (kt2 == nkt_q - 2)
                        q0 = QSB // 2 if last else 0
                        wsl = slice(q0, QSB)
                        deltas = [(kt2 + j) * KT - qs * QSB for j in range(2)]
                        q0s = [max(q0, min(d, QSB)) for d in deltas]
                        sps2 = pbig.tile([128, 2, QSB], F32, tag="big")
                        for j in range(2):
                            kt = kt2 + j
                            ksl = slice(kt * KT, (kt + 1) * KT)
                            jsl = slice(q0s[j], QSB)
                            nc.tensor.matmul(sps2[:, j, jsl], kt_h[:, ksl],
                                             qt_h[:, qs * QSB + q0s[j]:(qs + 1) * QSB],
                                             start=True, stop=True)
                        pt2 = pt_pool.tile([128, 2, QSB], BF16)
                        if q0s[0] == q0s[1]:
                            nc.scalar.activation(
                                out=pt2[:, :, wsl], in_=sps2[:, :, wsl],
                                func=mybir.ActivationFunctionType.Exp,
                                scale=float(HEAD_DIM) ** -0.5,
                            )
                        else:
                            for j in range(2):
                                jsl = slice(q0s[j], QSB)
                                nc.scalar.activation(
                                    out=pt2[:, j, jsl], in_=sps2[:, j, jsl],
                                    func=mybir.ActivationFunctionType.Exp,
                                    scale=float(HEAD_DIM) ** -0.5,
                                )
                        for j in range(2):
                            delta = deltas[j]
                            if delta >= -KT + 1:
                                jsl = slice(q0s[j], QSB)
                                # keep P[i, idx] iff (q0s+idx) - i - delta >= 0
                                nc.gpsimd.affine_select(
                                    out=pt2[:, j, jsl], in_=pt2[:, j, jsl],
                                    compare_op=mybir.AluOpType.is_ge,
                                    fill=0.0, base=q0s[j] - delta,
                                    channel_multiplier=-1,
                                    pattern=[[1, QSB - q0s[j]]],
                                )
                        for j in range(2):
                            kt = kt2 + j
                            jsl = slice(q0s[j], QSB)
                            nc.tensor.matmul(yps[:, jsl], Vsb[:, kt, h, :],
                                             pt2[:, j, jsl],
                                             start=(kt == 0),
                                             stop=(kt == nkt_q - 1))

                def finish_gen(h, qs, yps):
                    """transpose + normalize Y^T for (h, qs)."""
                    ysb = ysb_pool.tile([65, QSB], BF16)
                    nc.vector.tensor_copy(out=ysb, in_=yps)
                    yield
                    for qt in range(QSB // 128):
                        csl = slice(qs * QSB + qt * 128, qs * QSB + (qt + 1) * 128)
                        pt1 = paux.tile([128, 65], BF16, tag="aux")
                        nc.tensor.transpose(
                            pt1, ysb[:, qt * 128:(qt + 1) * 128], identb[0:65, 0:65])
                        rr = r_pool.tile([128, 1], F32)
                        nc.vector.reciprocal(rr, pt1[:, 64:65])
                        yqn = yqn_pool.tile([128, 64], BF16)
                        nc.vector.tensor_scalar_mul(yqn, pt1[:, 0:64], rr)
                        pt2r = paux.tile([64, 128], BF16, tag="aux")
                        nc.tensor.transpose(pt2r, yqn, identb)
                        if h == 0:
                            dst = YnT01[0:64, csl]
                        elif h == 1:
                            dst = YnT01[64:128, csl]
                        else:
                            dst = YnT2[0:64, csl]
                        nc.vector.tensor_copy(out=dst, in_=pt2r)
                        yield

                # ---------- interleaved schedule ----------
                side = deque()     # FIFO of generators (b/b2/d work)
                bwork = {}         # ch -> [gens] that must be emitted before
                                   # attention touches chunk ch

                def pull(n=1):
                    for _ in range(n):
                        while side:
                            try:
                                next(side[0])
                                break
                            except StopIteration:
                                side.popleft()
                        else:
                            return

                def drain(gens):
                    for g in gens:
                        for _ in g:
                            pass

                def drain_bwork_through(ch):
                    for c in range(ch + 1):
                        for g in bwork.pop(c, ()):
                            # may already be partially consumed via `side`
                            for _ in g:
                                pass

                def rotate(gens, pulls=1):
                    live = list(gens)
                    while live:
                        nxt = []
                        for g in live:
                            try:
                                next(g)
                                nxt.append(g)
                            except StopIteration:
                                pass
                            pull(pulls)
                        live = nxt

                has = lambda p: p in phases
                # prologue: chunk 0 must be ready before attention qs=0
                if has('B'):
                    drain([b_chunk_gen(0, split_dma=True)])
                if has('B2'):
                    drain([b2_gen(0)])

                if not has('C'):
                    for ch in range(1, n_qsb):
                        if has('B'):
                            drain([b_chunk_gen(ch)])
                        if has('B2'):
                            drain([b2_gen(ch)])
                    if has('D'):
                        for qs in range(n_qsb):
                            drain([d_gen(qs)])
                    continue

                fin2_prev = None   # finish gen of head 2 from previous qs
                for qs in range(n_qsb):
                    if qs + 1 < n_qsb:
                        gens = ([b_chunk_gen(qs + 1)] if has('B') else []) + \
                               ([b2_gen(qs + 1)] if has('B2') else [])
                        bwork[qs + 1] = gens
                        side.extend(gens)
                    # attention qs needs chunks <= qs fully emitted
                    drain_bwork_through(qs)

                    nkt_q = (qs + 1) * kt_per_qsb
                    yps0 = py_pool.tile([65, QSB], F32, tag="y", name=f"yps0_{qs}")
                    yps1 = py_pool.tile([65, QSB], F32, tag="y", name=f"yps1_{qs}")
                    g0 = attend_kloop_gen(0, qs, nkt_q, yps0)
                    g1 = attend_kloop_gen(1, qs, nkt_q, yps1)
                    rot_a = ([fin2_prev] if fin2_prev is not None else []) + [g0, g1]
                    rotate(rot_a, pulls=1)

                    if qs >= 1 and has('D'):
                        side.append(d_gen(qs - 1))
                    yps2 = py_pool.tile([65, QSB], F32, tag="y", name=f"yps2_{qs}")
                    g2 = attend_kloop_gen(2, qs, nkt_q, yps2)
                    f0 = finish_gen(0, qs, yps0)
                    f1 = finish_gen(1, qs, yps1)
                    rotate([f0, f1, g2], pulls=2)
                    fin2_prev = finish_gen(2, qs, yps2)

                # epilogue: lockstep head-2's last finish with the last
                # projection block (d tt-k needs fin2's qt-k written first)
                if not has('D'):
                    drain([fin2_prev])
                    pull(10 ** 9)
                    continue
                f, dg = fin2_prev, d_gen(n_qsb - 1)
                next(f)   # ysb copy
                next(f)   # qt0
                for _k in range(QSB // 128):
                    next(dg)              # tt-k first half (reads qt-k cols)
                    try:
                        next(f)           # qt-(k+1)
                    except StopIteration:
                        pass
                    next(dg)              # tt-k second half
                next(dg)                  # batched y DMA
                pull(10 ** 9)

    nc.compile()
    return nc


def make_in_maps(x, w_qkv, w_proj, T=T_FULL):
    """Per-core input dicts from full inputs (numpy), bf16-converted."""
    x = np.asarray(x, dtype=np.float32)
    w_qkv = np.asarray(w_qkv, dtype=np.float32).astype(NP_BF16)
    w_proj = np.asarray(w_proj, dtype=np.float32).astype(NP_BF16)
    cores_per_batch = N_CORES // B
    # x^T swizzled to [p, chunk, cchunk, tok] so each chunk DMA is one
    # contiguous run per partition (see XT in build_nc)
    n_ch = T // TOK_CHUNK
    xt_b = []
    for b in range(B):
        xt = x[b].T.reshape(CCHUNKS, 128, n_ch, TOK_CHUNK)
        xt_b.append(np.ascontiguousarray(
            xt.transpose(1, 2, 0, 3)).astype(NP_BF16))
    in_maps = []
    for core in range(N_CORES):
        b = core // cores_per_batch
        h0 = (core % cores_per_batch) * HEADS_PER_CORE
        h1, h2 = h0 + 1, h0 + 2
        col = lambda kind, h: w_qkv[:, kind * N_EMBD + h * HEAD_DIM:
                                    kind * N_EMBD + (h + 1) * HEAD_DIM]
        in_maps.append({
            "xt": xt_b[b],
            "wq01": np.ascontiguousarray(np.concatenate([col(0, h0), col(0, h1)], axis=1)),
            "wk01": np.ascontiguousarray(np.concatenate([col(1, h0), col(1, h1)], axis=1)),
            "wv01": np.ascontiguousarray(np.concatenate([col(2, h0), col(2, h1)], axis=1)),
            "wqv2": np.ascontiguousarray(np.concatenate([col(0, h2), col(2, h2)], axis=1)),
            "wk2": np.ascontiguousarray(col(1, h2)),
            "wp1": np.ascontiguousarray(w_proj[h0 * HEAD_DIM:(h1 + 1) * HEAD_DIM, :]),
            "wp2": np.ascontiguousarray(w_proj[h2 * HEAD_DIM:(h2 + 1) * HEAD_DIM, :]),
        })
    return in_maps


def gather_output(results, T=T_FULL):
    cores_per_batch = N_CORES // B
    out = np.empty((B, T, N_EMBD), dtype=np.float32)
    for b in range(B):
        parts = [np.asarray(results[b * cores_per_batch + j]["y"], dtype=np.float32)
                 for j in range(cores_per_batch)]
        out[b] = parts[0] + parts[1] + parts[2] + parts[3]
    return out


_CACHE = {}


def _get_nc(T=T_FULL, repeat=1):
    key = (T, repeat)
    if key not in _CACHE:
        _CACHE[key] = build_nc(T, repeat)
    return _CACHE[key]


def kernel(x, w_qkv, w_proj):
    import time as _time
    from concourse.bass_utils import run_bass_kernel_spmd
    T = x.shape[1]
    nc = _get_nc(T)
    in_maps = make_in_maps(x, w_qkv, w_proj, T)
    last_err = None
    for attempt in range(3):
        try:
            res = run_bass_kernel_spmd(nc, in_maps, list(range(N_CORES)))
            return gather_output(res.results, T)
        except Exception as e:  # transient device wedge: retry after a pause
            last_err = e
            _time.sleep(20 * (attempt + 1))
    raise last_err

